# revision 1
# baseline (speedup 1.0000x reference)
"""AttentionBlock (GroupNorm + MHA + proj + residual) on 8 trn2 NeuronCores.

Sharding: core = (batch b, L-half lh); x rolled so local queries are cols
0..1024 (softmax/groupnorm permutation-invariant over L -> all 8 cores run the
same graph, zero collectives).

v2 pipeline, built to saturate the Scalar engine (softmax exp is the hard
floor at ~147us/core):
  - scores N=1024 matmuls, head-pair row-group concurrency
  - exp (128,1024) f32 PSUM -> bf16 SBUF, double-buffered st tiles
  - AV packed into one (128,1024) PSUM accumulator via col tiling
  - denominators as 4 quarter-rows {0,32,64,96} of ONE PSUM bank
  - divide: reciprocal_approx_fast per quarter + f32 select-matmul broadcast
  - V-first QKV ordering; K/Q for later pairs interleaved into attention
  - groupnorm apply on the Scalar engine (per-partition scale/bias APs)
PSUM ledger: st 2x2 banks + av 2 + dn 1 + aux(rb/kq) 1 = 8 banks.
"""

import sys

for _p in ("/opt/trn_rl_repo", "/root/.axon_site/_ro/trn_rl_repo"):
    if _p not in sys.path:
        sys.path.insert(0, _p)

import numpy as np
import ml_dtypes

import concourse.bass as bass
import concourse.bacc as bacc
import concourse.tile as tile
from concourse import mybir

C = 512          # channels
L = 2048         # sequence length
LH = 1024        # local query half
B = 4            # batch
H = 8            # heads
D = 64           # head dim
G = 8            # groups
EPS = 1e-5
NT = C // 128    # channel tiles (4)
NKT = L // 128   # key-position tiles (16)
SCALE = D ** -0.5

f32 = mybir.dt.float32
i32 = mybir.dt.int32
bf16 = mybir.dt.bfloat16
AF = mybir.ActivationFunctionType
ALU = mybir.AluOpType

# Schraudolph exp constants (exp(SCALE*x) via the fp32 int bit trick), used
# when OFFLOAD_KT is non-empty to run part of softmax exp on the Vector engine.
A_EXP = float(1 << 23) / np.log(2.0) * SCALE
B_EXP = float(127 << 23) - 366393.0
# (pair, kt) whose qh=1 exp runs on DVE (Schraudolph) instead of ACT
OFFLOAD = frozenset((pr, kt) for pr in (1, 2, 3)
                    for kt in (5, 11))


def build_graph(debug=False):
    nc = bacc.Bacc(None, target_bir_lowering=False)

    x_e = nc.declare_dram_parameter("x", [C, L], f32, isOutput=False)
    w_e = nc.declare_dram_parameter("wqkvt", [C, 3 * C], bf16, isOutput=False)
    pw_e = nc.declare_dram_parameter("pwt", [C, C], bf16, isOutput=False)
    vecs_e = nc.declare_dram_parameter("vecs", [C, 8], f32, isOutput=False)
    indt_e = nc.declare_dram_parameter("indt", [NT, G, 128], f32, isOutput=False)
    out_e = nc.declare_dram_parameter("out", [C, LH], f32, isOutput=True)
    if debug:
        dbg_h_e = nc.declare_dram_parameter("dbg_h", [C, L], f32, isOutput=True)
        dbg_k_e = nc.declare_dram_parameter("dbg_k", [C, L], f32, isOutput=True)
        dbg_q_e = nc.declare_dram_parameter("dbg_q", [C, LH], f32, isOutput=True)
        dbg_v_e = nc.declare_dram_parameter("dbg_v", [128, C], f32, isOutput=True)
        dbg_a_e = nc.declare_dram_parameter("dbg_a", [C, LH], f32, isOutput=True)
        dbg_dn_e = nc.declare_dram_parameter("dbg_dn", [4, 512], f32, isOutput=True)
        dbg_rq_e = nc.declare_dram_parameter("dbg_rq", [4, 512], f32, isOutput=True)
        dbg_un_e = nc.declare_dram_parameter("dbg_un", [128, LH], f32, isOutput=True)

    with tile.TileContext(nc) as tc:
        with (
            tc.tile_pool(name="cst", bufs=1) as cst,
            tc.tile_pool(name="big", bufs=1) as big,
            tc.tile_pool(name="sm", bufs=2) as sm,
            tc.tile_pool(name="pp", bufs=3) as pp,
            tc.tile_pool(name="unp", bufs=2) as unp,
            tc.tile_pool(name="op", bufs=2) as op,
            tc.tile_pool(name="ps", bufs=1, space="PSUM") as ps,
        ):
            # ---- persistent SBUF tensors -------------------------------
            x_t = [big.tile([128, L], f32, name=f"x{t}", tag=f"x{t}")
                   for t in range(NT)]
            h_t = [big.tile([128, L], bf16, name=f"h{t}", tag=f"h{t}")
                   for t in range(NT)]
            k_t = [big.tile([128, L], bf16, name=f"k{t}", tag=f"k{t}")
                   for t in range(NT)]
            q_t = [big.tile([128, LH], bf16, name=f"q{t}", tag=f"q{t}")
                   for t in range(NT)]
            vT1 = [big.tile([128, C], bf16, name=f"v{t}", tag=f"v{t}")
                   for t in range(NKT)]
            attn_t = [big.tile([128, LH], bf16, name=f"a{t}", tag=f"a{t}")
                      for t in range(NT)]
            w_t = [big.tile([128, 3 * C], bf16, name=f"w{t}", tag=f"w{t}")
                   for t in range(NT)]
            pw_t = [big.tile([128, C], bf16, name=f"pw{t}", tag=f"pw{t}")
                    for t in range(NT)]
            # reciprocal scratch: rows {0,32,64,96} hold recs, rest preset 1.0
            recq = [big.tile([97, 512], f32, name=f"recq{i}", tag=f"recq{i}")
                    for i in range(2)]
            # SBUF staging for the PSUM denominator rows (custom-DVE recip
            # cannot read PSUM)
            dnc = [big.tile([97, 512], f32, name=f"dnc{i}", tag=f"dnc{i}")
                   for i in range(2)]
            # select matrices for the rb broadcast matmuls (bf16: fp32
            # matmuls are self-loading with a single ISA wait slot)
            rbsel = [big.tile([97, 128], bf16, name=f"rbsel{h}", tag=f"rbsel{h}")
                     for h in range(2)]

            # x on the sync queue, weights on the scalar queue, small vectors
            # on the vector queue -- three DMA queues run concurrently
            vecs_t = [cst.tile([128, 8], f32, name=f"vecs{t}", tag=f"vecs{t}")
                      for t in range(NT)]
            for t in range(NT):
                nc.gpsimd.dma_start(
                    out=vecs_t[t], in_=vecs_e[t * 128:(t + 1) * 128, :]
                )
            xq = [nc.sync, nc.scalar, nc.gpsimd, nc.sync]
            for t in range(NT):
                xq[t].dma_start(out=x_t[t], in_=x_e[t * 128:(t + 1) * 128, :])
            for t in range(NT):
                nc.scalar.dma_start(out=w_t[t], in_=w_e[t * 128:(t + 1) * 128, :])
            nw_t = [vecs_t[t][:, 0:1] for t in range(NT)]
            nb_t = [vecs_t[t][:, 1:2] for t in range(NT)]
            qb_t = [vecs_t[t][:, 2:3] for t in range(NT)]
            kb_t = [vecs_t[t][:, 3:4] for t in range(NT)]
            pbe_t = [vecs_t[t][:, 4:5] for t in range(NT)]
            for t in range(NT):
                nc.scalar.dma_start(out=pw_t[t], in_=pw_e[t * 128:(t + 1) * 128, :])

            eps_t = cst.tile([G, 1], f32, name="eps", tag="eps")
            nc.vector.memset(eps_t, EPS)
            wu_a = cst.tile([128, 128], bf16, name="wu_a", tag="wu_a")
            nc.vector.memset(wu_a, 0.5)
            wu_b = cst.tile([128, 512], bf16, name="wu_b", tag="wu_b")
            nc.vector.memset(wu_b, 0.5)
            for wi in range(24):
                wups = ps.tile([128, 512], f32, name=f"wu{wi}", tag="aux")
                nc.tensor.matmul(wups, wu_a, wu_b, start=True, stop=True)
            ones1 = cst.tile([128, 1], bf16, name="ones1", tag="ones1")
            nc.vector.memset(ones1, 1.0)
            for i in range(2):
                nc.vector.memset(recq[i], 1.0)
                nc.vector.memset(dnc[i], 1.0)
            for h in range(2):
                nc.vector.memset(rbsel[h], 0.0)
                nc.vector.memset(rbsel[h][32 * h:32 * h + 1, 0:64], 1.0)
                nc.vector.memset(rbsel[h][64 + 32 * h:65 + 32 * h, 64:128], 1.0)

            # group indicator matrices for cross-partition stats
            ind = [cst.tile([128, G], bf16, name=f"ind{t}", tag=f"ind{t}")
                   for t in range(NT)]
            indT = [cst.tile([G, 128], bf16, name=f"indT{t}", tag=f"indT{t}")
                    for t in range(NT)]
            indT_f = [cst.tile([G, 128], f32, name=f"indTf{t}", tag=f"indTf{t}")
                      for t in range(NT)]
            for t in range(NT):
                nc.vector.memset(ind[t], 0.0)
                nc.vector.memset(ind[t][0:64, 2 * t:2 * t + 1], 1.0 / D)
                nc.vector.memset(ind[t][64:128, 2 * t + 1:2 * t + 2], 1.0 / D)
                nc.gpsimd.dma_start(out=indT_f[t], in_=indt_e[t])
                nc.vector.tensor_copy(out=indT[t], in_=indT_f[t])

            # ---- groupnorm stats: tiles 0/1 on DVE (bn_stats), tiles 2/3
            # on ACT (Copy/Square with accum_out), running concurrently
            stats2 = [None] * NT
            for t in (0, 1):
                bn = sm.tile([128, L // 512, 6], f32, name="bn", tag="bn")
                for s in range(L // 512):
                    nc.vector.bn_stats(
                        out=bn[:, s, :], in_=x_t[t][:, s * 512:(s + 1) * 512]
                    )
                mv = sm.tile([128, 2], f32, name=f"mv{t}", tag=f"mv{t}")
                nc.vector.bn_aggr(out=mv, in_=bn)
                s2 = sm.tile([128, 2], bf16, name=f"s2{t}", tag=f"s2{t}")
                nc.vector.tensor_copy(out=s2[:, 0:1], in_=mv[:, 0:1])
                nc.vector.tensor_mul(s2[:, 1:2], mv[:, 0:1], mv[:, 0:1])
                nc.vector.tensor_add(s2[:, 1:2], s2[:, 1:2], mv[:, 1:2])
                stats2[t] = s2
            sdump = big.tile([128, L], f32, name="sdump", tag="sdump")
            for t in (2, 3):
                acc = sm.tile([128, 2], f32, name=f"acc{t}", tag=f"acc{t}")
                nc.scalar.activation(
                    out=sdump, in_=x_t[t],
                    func=AF.Identity, accum_out=acc[:, 0:1],
                )
                nc.scalar.activation(
                    out=sdump, in_=x_t[t],
                    func=AF.Square, accum_out=acc[:, 1:2],
                )
                s2 = sm.tile([128, 2], bf16, name=f"s2{t}", tag=f"s2{t}")
                nc.vector.tensor_scalar(
                    out=s2, in0=acc, scalar1=1.0 / L, scalar2=None,
                    op0=ALU.mult,
                )
                stats2[t] = s2

            gps = ps.tile([G, 2], f32, name="gps", tag="aux")
            for t in range(NT):
                nc.tensor.matmul(
                    gps, ind[t], stats2[t], start=(t == 0), stop=(t == NT - 1)
                )
            mean_g = sm.tile([G, 1], f32, name="mean_g", tag="mean_g")
            nc.vector.tensor_copy(out=mean_g, in_=gps[:, 0:1])
            var_g = sm.tile([G, 1], f32, name="var_g", tag="var_g")
            nc.vector.tensor_mul(var_g, mean_g, mean_g)
            nc.vector.tensor_sub(var_g, gps[:, 1:2], var_g)
            gsb = sm.tile([G, 2], bf16, name="gsb", tag="gsb")
            nc.vector.tensor_copy(out=gsb[:, 0:1], in_=mean_g)
            std_g = sm.tile([G, 1], f32, name="std_g", tag="std_g")
            nc.scalar.activation(
                out=std_g, in_=var_g, func=AF.Sqrt, bias=eps_t, scale=1.0
            )
            with nc.allow_low_precision(reason="groupnorm rstd in bf16"):
                nc.vector.reciprocal(out=gsb[:, 1:2], in_=std_g)

            A_t, B_t = [], []
            for t in range(NT):
                bc = ps.tile([128, 2], f32, name="bc", tag="aux")
                nc.tensor.matmul(bc, indT[t], gsb, start=True, stop=True)
                A = sm.tile([128, 1], f32, name=f"A{t}", tag=f"A{t}")
                Bt = sm.tile([128, 1], f32, name=f"Bt{t}", tag=f"Bt{t}")
                nc.vector.tensor_mul(A, nw_t[t], bc[:, 1:2])
                nc.vector.tensor_mul(Bt, bc[:, 0:1], A)
                nc.vector.tensor_sub(Bt, nb_t[t], Bt)
                A_t.append(A)
                B_t.append(Bt)

            # ---- QKV matmul helpers ------------------------------------
            def emit_v(lt, tag="st"):
                vps = ps.tile([128, 512], f32, name=f"vps{lt}", tag=tag,
                              bufs=2 if tag == "st" else 1)
                for ct in range(NT):
                    nc.tensor.matmul(
                        vps,
                        h_t[ct][:, lt * 128:(lt + 1) * 128],
                        w_t[ct][:, 2 * C:3 * C],
                        start=(ct == 0), stop=(ct == NT - 1),
                    )
                nc.vector.tensor_copy(out=vT1[lt], in_=vps)

            def emit_k(pr, nk, tag="st"):
                kps = ps.tile([128, 512], f32, name=f"kps{pr}{nk}", tag=tag, bufs=2 if tag == "st" else 1)
                for ct in range(NT):
                    nc.tensor.matmul(
                        kps,
                        w_t[ct][:, C + pr * 128:C + (pr + 1) * 128],
                        h_t[ct][:, nk * 512:(nk + 1) * 512],
                        start=(ct == 0), stop=(ct == NT - 1),
                    )
                nc.vector.tensor_scalar(
                    out=k_t[pr][:, nk * 512:(nk + 1) * 512], in0=kps,
                    scalar1=kb_t[pr], scalar2=None, op0=ALU.add,
                )

            def emit_q(pr, nq, tag="st"):
                qps = ps.tile([128, 512], f32, name=f"qps{pr}{nq}", tag=tag, bufs=2 if tag == "st" else 1)
                for ct in range(NT):
                    nc.tensor.matmul(
                        qps,
                        w_t[ct][:, pr * 128:(pr + 1) * 128],
                        h_t[ct][:, nq * 512:(nq + 1) * 512],
                        start=(ct == 0), stop=(ct == NT - 1),
                    )
                nc.vector.tensor_scalar(
                    out=q_t[pr][:, nq * 512:(nq + 1) * 512], in0=qps,
                    scalar1=qb_t[pr], scalar2=None, op0=ALU.add,
                )

            # h apply chunk-major, split across ACT and DVE; only the work
            # pair-0's first kts need runs before attention, the rest is
            # spread into the attention loop
            def emit_h(s):
                for t in range(NT):
                    if (s + t) % 2 == 0:
                        nc.scalar.activation(
                            out=h_t[t][:, s * 512:(s + 1) * 512],
                            in_=x_t[t][:, s * 512:(s + 1) * 512],
                            func=AF.Identity, bias=B_t[t], scale=A_t[t],
                        )
                    else:
                        nc.vector.tensor_scalar(
                            out=h_t[t][:, s * 512:(s + 1) * 512],
                            in0=x_t[t][:, s * 512:(s + 1) * 512],
                            scalar1=A_t[t], scalar2=B_t[t],
                            op0=ALU.mult, op1=ALU.add,
                        )

            for s in range(2):
                emit_h(s)
                emit_k(0, s)
                emit_q(0, s)
            emit_v(0)
            emit_v(1)

            # remaining V / K / Q work, spread into the attention loop with
            # explicit deadlines: v[lt] is emitted 6 kts before its use; all
            # of K/Q for pair pr+1 is emitted before pair pr ends.
            spread = {}
            for lt in range(2, NKT):
                spread.setdefault((0, lt - 2), []).append(("v", lt, 0))
            spread.setdefault((0, 1), []).append(("h", 2, 0))
            spread.setdefault((0, 3), []).append(("h", 3, 0))
            spread.setdefault((0, 2), []).append(("k", 0, 2))
            spread.setdefault((0, 4), []).append(("k", 0, 3))
            kq_list = []
            for pr in range(1, NT):
                for nk in range(4):
                    kq_list.append(("k", pr, nk))
                for nq in range(2):
                    kq_list.append(("q", pr, nq))
            slots = ([(0, kt) for kt in range(5, 11)]
                     + [(1, kt) for kt in range(2, 8)]
                     + [(2, kt) for kt in range(2, 8)])
            for slot, ent in zip(slots, kq_list):
                spread.setdefault(slot, []).append(ent)

            # ---- attention ---------------------------------------------
            def emit_divide_pre(pr, dn, rq, dc):
                # stage the PSUM denominators out + reciprocal, at pair end so
                # the dn bank is fully read before its next-pair reuse
                for r in (0, 32, 64, 96):
                    nc.vector.tensor_copy(out=dc[r:r + 1, :], in_=dn[r:r + 1, :])
                nc.vector.reciprocal_approx_fast(out=rq, in_=dc)
                rqb = sm.tile([97, 512], bf16, name=f"rqb{pr}", tag="rqb")
                nc.vector.tensor_copy(out=rqb, in_=rq)
                return rqb

            def emit_divide(pr, unn, rqb):
                for hh in range(2):
                    rb = ps.tile([128, 512], f32, name=f"rb{pr}{hh}", tag="aux")
                    nc.tensor.matmul(rb, rbsel[hh], rqb, start=True, stop=True)
                    nc.vector.tensor_mul(
                        attn_t[pr][:, hh * 512:(hh + 1) * 512],
                        unn[:, hh * 512:(hh + 1) * 512], rb,
                    )

            # flat sub-iteration schedule: i = (pr, kt, qh); scores/exp of
            # sub-iter i are emitted together, av/dn of i-1 follow (1-deep
            # software pipeline keeps the next scores off the exp gate)
            av_t = {}
            dn_t = {}
            subs = [(pr, kt, qh)
                    for pr in range(NT) for kt in range(NKT) for qh in range(2)]

            def emit_scores(i):
                pr, kt, qh = subs[i]
                st = ps.tile([128, LH], f32, name=f"st{pr}{kt}{qh}",
                             tag="st", bufs=2)
                p = pp.tile([128, LH], bf16, name=f"p{pr}{kt}{qh}", tag="p",
                            bufs=4)
                for j in range(2):
                    hp0 = j * 64
                    nc.tensor.matmul(
                        st[:, j * 512:(j + 1) * 512],
                        k_t[pr][hp0:hp0 + 64, kt * 128:(kt + 1) * 128],
                        q_t[pr][hp0:hp0 + 64, qh * 512:(qh + 1) * 512],
                        start=True, stop=True,
                    )
                if qh == 1 and (pr, kt) in OFFLOAD:
                    ti = unp.tile([128, LH], i32, name=f"ti{pr}{kt}", tag="ti")
                    nc.vector.tensor_scalar(
                        out=ti, in0=st, scalar1=A_EXP, scalar2=B_EXP,
                        op0=ALU.mult, op1=ALU.add,
                    )
                    nc.vector.tensor_copy(out=p, in_=ti.bitcast(f32))
                else:
                    nc.scalar.activation(out=p, in_=st, func=AF.Exp,
                                         scale=SCALE)
                return p

            def emit_avdn(i, p):
                pr, kt, qh = subs[i]
                if kt == 0 and qh == 0:
                    av_t[pr] = ps.tile([128, LH], f32, name=f"av{pr}",
                                       tag="av")
                    dn_t[pr] = ps.tile([97, 512], f32, name=f"dn{pr}",
                                       tag="dn")
                av, dn = av_t[pr], dn_t[pr]
                first = (kt == 0)
                last = (kt == NKT - 1)
                for j in range(2):
                    h2 = 2 * pr + j
                    nc.tensor.matmul(
                        av[64 * j:64 * j + 64, qh * 512:(qh + 1) * 512],
                        vT1[kt][:, h2 * D:(h2 + 1) * D],
                        p[:, j * 512:(j + 1) * 512],
                        start=first, stop=last,
                        tile_position=(0, 64 * j),
                    )
                for j in range(2):
                    r = 32 * qh + 64 * j
                    nc.tensor.matmul(
                        dn[r:r + 1, :], ones1, p[:, j * 512:(j + 1) * 512],
                        start=first, stop=last,
                        tile_position=(0, r),
                    )
                if last and qh == 1:
                    # release the packed accumulator, stage divide inputs
                    unn = unp.tile([128, LH], f32, name=f"unn{pr}", tag="unn")
                    nc.vector.tensor_copy(out=unn, in_=av)
                    rqb = emit_divide_pre(pr, dn, recq[pr % 2], dnc[pr % 2])
                    return (pr, unn, rqb)
                return None

            pq = []  # (index, p) awaiting av/dn, 2-deep
            pending = None
            for i in range(len(subs)):
                pr, kt, qh = subs[i]
                p = emit_scores(i)
                pq.append((i, p))
                if len(pq) > 2:
                    io, po = pq.pop(0)
                    done = emit_avdn(io, po)
                    if done is not None:
                        pending = done
                # divide of the previous pair, a few sub-iters into this one
                if pending is not None and kt == 1 and qh == 1:
                    emit_divide(*pending)
                    pending = None
                # spread remaining V/K/Q matmul groups at their deadlines
                if qh == 1:
                    for kind, wpr, wn in spread.get((pr, kt), ()):
                        if kind == "v":
                            emit_v(wpr, tag="aux")
                        elif kind == "k":
                            emit_k(wpr, wn, tag="aux")
                        elif kind == "h":
                            emit_h(wpr)
                        else:
                            emit_q(wpr, wn, tag="aux")
            for io, po in pq:
                done = emit_avdn(io, po)
                if done is not None:
                    pending = done
            if debug:
                dbg_unn = big.tile([128, LH], f32, name="dbg_unn",
                                   tag="dbg_unn")
                nc.vector.tensor_copy(out=dbg_unn, in_=pending[1])
                ddn = big.tile([97, 512], f32, name="ddn", tag="ddn")
                for r in (0, 32, 64, 96):
                    nc.vector.tensor_copy(out=ddn[r:r + 1, :],
                                          in_=pending[2][r:r + 1, :])
                for i_, r in enumerate((0, 32, 64, 96)):
                    nc.sync.dma_start(out=dbg_dn_e[i_:i_ + 1, :],
                                      in_=ddn[r:r + 1, :])
            emit_divide(*pending)

            if debug:
                for t in range(NT):
                    for s_ in range(4):
                        dcp = op.tile([128, 512], f32, name="dcp", tag="dcp")
                        nc.vector.tensor_copy(out=dcp, in_=h_t[t][:, s_*512:(s_+1)*512])
                        nc.sync.dma_start(out=dbg_h_e[t*128:(t+1)*128, s_*512:(s_+1)*512], in_=dcp)
                        dck = op.tile([128, 512], f32, name="dck", tag="dcp")
                        nc.vector.tensor_copy(out=dck, in_=k_t[t][:, s_*512:(s_+1)*512])
                        nc.sync.dma_start(out=dbg_k_e[t*128:(t+1)*128, s_*512:(s_+1)*512], in_=dck)
                    for s_ in range(2):
                        dcq = op.tile([128, 512], f32, name="dcq", tag="dcp")
                        nc.vector.tensor_copy(out=dcq, in_=q_t[t][:, s_*512:(s_+1)*512])
                        nc.sync.dma_start(out=dbg_q_e[t*128:(t+1)*128, s_*512:(s_+1)*512], in_=dcq)
                        dca = op.tile([128, 512], f32, name="dca", tag="dcp")
                        nc.vector.tensor_copy(out=dca, in_=attn_t[t][:, s_*512:(s_+1)*512])
                        nc.sync.dma_start(out=dbg_a_e[t*128:(t+1)*128, s_*512:(s_+1)*512], in_=dca)
                dcv = op.tile([128, 512], f32, name="dcv", tag="dcp")
                nc.vector.tensor_copy(out=dcv, in_=vT1[0])
                nc.sync.dma_start(out=dbg_v_e[:, :], in_=dcv)
                for i, r in enumerate((0, 32, 64, 96)):
                    nc.sync.dma_start(out=dbg_rq_e[i:i+1, :], in_=recq[1][r:r+1, :])
                nc.sync.dma_start(out=dbg_un_e[:, :], in_=dbg_unn)

            # ---- proj + residual + store -------------------------------
            for hh in range(2):
                for mo in range(NT):
                    pj = ps.tile([128, 512], f32, name=f"pj{hh}{mo}", tag="st", bufs=2)
                    for ct in range(NT):
                        nc.tensor.matmul(
                            pj,
                            pw_t[ct][:, mo * 128:(mo + 1) * 128],
                            attn_t[ct][:, hh * 512:(hh + 1) * 512],
                            start=(ct == 0), stop=(ct == NT - 1),
                        )
                    o = op.tile([128, 512], f32, name=f"o{hh}{mo}", tag="o")
                    nc.vector.scalar_tensor_tensor(
                        out=o, in0=pj, scalar=pbe_t[mo],
                        in1=x_t[mo][:, hh * 512:(hh + 1) * 512],
                        op0=ALU.add, op1=ALU.add,
                    )
                    oq = (nc.sync, nc.scalar, nc.gpsimd)[(hh * NT + mo) % 3]
                    oq.dma_start(
                        out=out_e[mo * 128:(mo + 1) * 128,
                                  hh * 512:(hh + 1) * 512],
                        in_=o,
                    )
    nc.compile()
    return nc


_NC = None


def _get_nc():
    global _NC
    if _NC is None:
        _NC = build_graph()
    return _NC


def _make_in_maps(x, norm_w, norm_b, qkv_w, qkv_b, proj_w, proj_b):
    bfl = ml_dtypes.bfloat16
    wqkvt = np.ascontiguousarray(qkv_w.T.astype(bfl))
    pwt = np.ascontiguousarray(proj_w.T.astype(bfl))
    qb = np.ascontiguousarray(qkv_b[0:C].astype(np.float32))
    kb = np.ascontiguousarray(qkv_b[C:2 * C].astype(np.float32))
    vb = qkv_b[2 * C:3 * C].astype(np.float32)
    # v-bias folds into an effective proj bias (softmax rows sum to 1)
    pbe = np.ascontiguousarray(
        (proj_b.astype(np.float32) + proj_w.astype(np.float32) @ vb)
    )
    vecs = np.zeros((C, 8), dtype=np.float32)
    vecs[:, 0] = norm_w.astype(np.float32)
    vecs[:, 1] = norm_b.astype(np.float32)
    vecs[:, 2] = qb
    vecs[:, 3] = kb
    vecs[:, 4] = pbe

    indt = np.zeros((NT, G, 128), dtype=np.float32)
    for t in range(NT):
        indt[t, 2 * t, 0:64] = 1.0
        indt[t, 2 * t + 1, 64:128] = 1.0

    shared = {"wqkvt": wqkvt, "pwt": pwt, "vecs": vecs, "indt": indt}
    in_maps = []
    for core in range(8):
        b, lh = core // 2, core % 2
        xb = np.asarray(x[b], dtype=np.float32)
        if lh:
            xb = np.concatenate([xb[:, LH:], xb[:, :LH]], axis=1)
        m = dict(shared)
        m["x"] = np.ascontiguousarray(xb)
        in_maps.append(m)
    return in_maps


def run(inputs, trace=False, tmpdir=None):
    from concourse.bass_utils import run_bass_kernel_spmd

    nc = _get_nc()
    in_maps = _make_in_maps(**inputs)
    res = run_bass_kernel_spmd(
        nc, in_maps, core_ids=list(range(8)), trace=trace, tmpdir=tmpdir
    )
    out = np.empty((B, C, L), dtype=np.float32)
    for core in range(8):
        b, lh = core // 2, core % 2
        out[b, :, lh * LH:(lh + 1) * LH] = res.results[core]["out"]
    return out, res


def kernel(**inputs):
    out, _ = run(inputs, trace=False)
    return out

